# revision 19
# baseline (speedup 1.0000x reference)
"""Bass/Tile kernel builder for DeformConv (one sample per NeuronCore).

Index conventions:
  l = i * 128 + p   (p = SBUF partition, i = l-block 0..31)
  image row = l // 64, col = l % 64;  tap k = ky*3 + kx
  base arrays [128 p, 9 k, 32 i] fp32
  corner weights w4 [128, 4 r, 9 k, 32 i] fp32
  pix16 [128, 18 t, 32 i] int16, t = 2k + (0 top / 1 bottom pair)
  xtd DRAM [4224, 128] fp16, row = pixel + 64 (guard bands zeroed)
  gather t: list position j = i_local*128 + p -> idx[q=j%16, s=j//16=i_local*8+p//16]
"""
from contextlib import ExitStack

import numpy as np

import concourse.bass as bass
import concourse.mybir as mybir
import concourse.tile as tile
from concourse import masks
from concourse import dve_ops as _dve_ops


def _register_scale2_add():
    """Custom DVE op: out = in0*s0 + in1*s1 (two per-partition scalars).
    Does a corner-pair of the bilinear combine in one Vector instruction."""
    for op in _dve_ops.OPS:
        if op.name == "SCALE2_ADD_ANT":
            return op
    from concourse.dve_spec import Spec, Src0, Src1, C0, C1, lower
    from concourse.dve_uop import DveOpSpec

    spec = Spec(
        body=Src0 * C0 + Src1 * C1,
        reference=lambda in0, in1, s0, s1, imm2: (
            in0.astype(np.float32) * s0 + in1.astype(np.float32) * s1),
    )
    shas = {}
    for ver in ("v3", "v4"):
        s = DveOpSpec(name="SCALE2_ADD_ANT", opcode=0,
                      uops=lower(spec, ver=ver), rd1_en=True)
        shas[ver] = s.sha(ver)
    op = _dve_ops.DveOp("SCALE2_ADD_ANT", spec, subdim=False, uops_sha=shas)
    _dve_ops.OPS.append(op)
    _dve_ops._SUB_OPCODE_FOR_NAME[op.name] = (
        _dve_ops._CUSTOM_DVE_ROW_BASE + len(_dve_ops.OPS) - 1)
    _dve_ops.CUSTOM_DVE_SPECS[op.name] = spec
    return op


SCALE2 = _register_scale2_add()

F32 = mybir.dt.float32
F32R = mybir.dt.float32r
F16 = mybir.dt.float16
I16 = mybir.dt.int16
ALU = mybir.AluOpType
ACTF = mybir.ActivationFunctionType

Cin = Cout = 128
HW = 4096
NTAP = 9
GUARD = 64
NROWS = HW + 2 * GUARD
NBLK = 32
NHALF = 2
BPH = NBLK // NHALF  # blocks per half

# stage toggles for cost attribution (dev only)
CFG = {"gather": True, "combine": True, "transpose": True, "matmul": True}


def host_constants():
    l = np.arange(HW)
    p = l % 128
    blk = l // 128
    i_img = l // 64
    j_img = l % 64
    ky = np.arange(9) // 3
    kx = np.arange(9) % 3
    basepy = np.zeros((128, 9, 32), np.float32)
    basepx = np.zeros((128, 9, 32), np.float32)
    for k in range(9):
        basepy[p, k, blk] = i_img - 1 + ky[k]
        basepx[p, k, blk] = j_img - 1 + kx[k]
    return {"basepy": basepy, "basepx": basepx}


def host_weights(w_offset, w):
    wofft = np.ascontiguousarray(
        w_offset.transpose(2, 3, 1, 0).reshape(9, 128, 18)).astype(np.float16)
    wmainT = np.ascontiguousarray(
        w.transpose(2, 3, 1, 0).reshape(9, 128, 128)).astype(np.float16)
    return {"wofft": wofft, "wmainT": wmainT}


def declare_io(nc, debug=False):
    io = {}
    io["xin"] = nc.dram_tensor("xin", (128, HW), F32, kind="ExternalInput").ap()
    io["wofft"] = nc.dram_tensor("wofft", (9, 128, 18), F16, kind="ExternalInput").ap()
    io["wmainT"] = nc.dram_tensor("wmainT", (9, 128, 128), F16, kind="ExternalInput").ap()
    io["basepy"] = nc.dram_tensor("basepy", (128, 9, 32), F32, kind="ExternalInput").ap()
    io["basepx"] = nc.dram_tensor("basepx", (128, 9, 32), F32, kind="ExternalInput").ap()
    io["out"] = nc.dram_tensor("out", (128, HW), F32, kind="ExternalOutput").ap()
    io["pixb"] = nc.dram_tensor("pixb", (128, 9 * 32), I16,
                                kind="ExternalOutput" if debug else "Internal").ap()
    io["pixb_e"] = nc.dram_tensor("pixb_e", (128, 16), I16, kind="Internal").ap()
    io["xpair"] = nc.dram_tensor("xpair", (4352, 2, 128), F16, kind="Internal").ap()
    io["debug"] = debug
    if debug:
        io["d_offsb"] = nc.dram_tensor("d_offsb", (18, HW), F32, kind="ExternalOutput").ap()
        io["d_offT"] = nc.dram_tensor("d_offT", (128, 32 * 18), F32, kind="ExternalOutput").ap()
        io["d_w4"] = nc.dram_tensor("d_w4", (128, 4 * 9 * 32), F32, kind="ExternalOutput").ap()
        io["d_idxw"] = nc.dram_tensor("d_idxw", (128, 9 * 256), I16, kind="ExternalOutput").ap()
        io["d_gtop"] = nc.dram_tensor("d_gtop", (128, BPH * 256), F16, kind="ExternalOutput").ap()
        io["d_gbot"] = nc.dram_tensor("d_gbot", (128, BPH * 256), F16, kind="ExternalOutput").ap()
        io["d_sampT"] = nc.dram_tensor("d_sampT", (128, BPH * 128), F16, kind="ExternalOutput").ap()
        io["d_sampN"] = nc.dram_tensor("d_sampN", (128, BPH * 128), F16, kind="ExternalOutput").ap()
        for nm in ("d_py", "d_fy", "d_y0", "d_x0", "d_fx", "d_my0", "d_a0"):
            io[nm] = nc.dram_tensor(nm, (128, 32), F32, kind="ExternalOutput").ap()
    return io


def build(tc: tile.TileContext, io: dict):
    nc = tc.nc
    xin, wofft, wmainT = io["xin"], io["wofft"], io["wmainT"]
    basepy, basepx, out = io["basepy"], io["basepx"], io["out"]
    pixb, xpair = io["pixb"], io["xpair"]

    ctx = ExitStack()
    const = ctx.enter_context(tc.tile_pool(name="const", bufs=1))
    persist = ctx.enter_context(tc.tile_pool(name="persist", bufs=1))
    coord = ctx.enter_context(tc.tile_pool(name="coord", bufs=2))
    evac = ctx.enter_context(tc.tile_pool(name="evac", bufs=3))

    ident32 = const.tile([128, 128], F32)
    masks.make_identity(nc, ident32[:])
    ident16 = const.tile([128, 128], F16)
    masks.make_identity(nc, ident16[:])
    zeros16 = const.tile([128, 128], F16)
    nc.vector.memset(zeros16[:], 0.0)

    # zero the xpair cells no data write covers (guard bands)
    xpair_flat = xpair.rearrange("r s c -> (r s c)")
    for off, rows in (
        (0, 128), (256 * 4224, 128),             # slot0: r in [0,128)+[4224,4352)
        (128, 64),                               # slot1: r in [0,64)
        (256 * 4160 + 128, 128), (256 * 4288 + 128, 64),  # slot1: [4160,4352)
    ):
        nc.sync.dma_start(
            out=bass.AP(xpair_flat.tensor, off, [[256, rows], [1, 128]]),
            in_=zeros16[0:rows, :])

    xpad = persist.tile([128, 66, 66], F16)
    nc.vector.memset(xpad[:], 0.0)
    nc.gpsimd.dma_start(out=xpad[:, 1:65, 1:65],
                        in_=xin.rearrange("c (h w) -> c h w", h=64))
    # column-shifted contiguous copies: xsh[:, kx, r, j] = xpad[c, r, j+kx]
    xsh = persist.tile([128, 3, 66 * 64], F16)
    for kx in range(3):
        nc.vector.tensor_copy(
            xsh[:, kx, :].rearrange("p (r j) -> p r j", r=66),
            xpad[:, :, kx:kx + 64])

    wofft_sb = persist.tile([128, 9, 18], F16)
    nc.sync.dma_start(out=wofft_sb[:], in_=wofft.rearrange("k c f -> c k f"))
    wmainT_sb = persist.tile([128, 9, 128], F16)
    nc.sync.dma_start(out=wmainT_sb[:], in_=wmainT.rearrange("k c o -> c k o"))
    basepy_sb = persist.tile([128, 9, 32], F32)
    nc.sync.dma_start(out=basepy_sb[:], in_=basepy)
    basepx_sb = persist.tile([128, 9, 32], F32)
    nc.sync.dma_start(out=basepx_sb[:], in_=basepx)

    offsb = persist.tile([18, HW], F32)
    offT = persist.tile([128, 32, 18], F32)
    w4s = [persist.tile([128, 4, 32], F32, name=f"w4_{k}") for k in range(NTAP)]
    pix16 = persist.tile([128, 9, 32], I16)
    idxws = [persist.tile([128, 256], I16, name=f"idxw_{k}") for k in range(NTAP)]
    offT_e = persist.tile([128, 16, 18], F32)
    w4_e = persist.tile([128, 4, 16], F32)
    pix_e = persist.tile([128, 16], I16)
    idxw_e = persist.tile([128, 128], I16)

    def coord_block(dy, dx, bpy, bpx, w4dst, pixdst, ncol):
        """bilinear coords -> 4 corner weights + clamped pair-row index."""
        py = coord.tile([128, ncol], F32, tag=f"py{ncol}")
        nc.vector.tensor_tensor(py[:], dy, bpy, ALU.add)
        px = coord.tile([128, ncol], F32, tag=f"px{ncol}")
        nc.vector.tensor_tensor(px[:], dx, bpx, ALU.add)

        def floorfrac(s, tagp):
            ti = coord.tile([128, ncol], mybir.dt.int32, tag=tagp + "i")
            nc.vector.tensor_copy(ti[:], s[:])
            tf = coord.tile([128, ncol], F32, tag=tagp + "f")
            nc.vector.tensor_copy(tf[:], ti[:])
            gt = coord.tile([128, ncol], F32, tag=tagp + "g")
            nc.vector.tensor_tensor(gt[:], tf[:], s[:], ALU.is_gt)
            fl = coord.tile([128, ncol], F32, tag=tagp + "fl")
            nc.vector.tensor_tensor(fl[:], tf[:], gt[:], ALU.subtract)
            fr = coord.tile([128, ncol], F32, tag=tagp + "fr")
            nc.vector.tensor_tensor(fr[:], s[:], fl[:], ALU.subtract)
            return fl, fr

        y0, fy = floorfrac(py, f"yy{ncol}")
        x0, fx = floorfrac(px, f"xx{ncol}")

        def wmask(s, lo, hi, tag):
            m1 = coord.tile([128, ncol], F32, tag=tag + "a")
            nc.vector.tensor_scalar(m1[:], s[:], float(lo), None, ALU.is_ge)
            m2 = coord.tile([128, ncol], F32, tag=tag + "b")
            nc.vector.tensor_scalar(m2[:], s[:], float(hi), None, ALU.is_le)
            m = coord.tile([128, ncol], F32, tag=tag)
            nc.vector.tensor_tensor(m[:], m1[:], m2[:], ALU.mult)
            return m

        my0 = wmask(y0, 0, 63, f"my0{ncol}")
        my1 = wmask(y0, -1, 62, f"my1{ncol}")
        mx0 = wmask(x0, 0, 63, f"mx0{ncol}")
        mx1 = wmask(x0, -1, 62, f"mx1{ncol}")

        a0 = coord.tile([128, ncol], F32, tag=f"a0{ncol}")
        nc.vector.tensor_scalar(a0[:], fy[:], -1.0, 1.0, ALU.mult, ALU.add)
        nc.vector.tensor_tensor(a0[:], a0[:], my0[:], ALU.mult)
        a1 = coord.tile([128, ncol], F32, tag=f"a1{ncol}")
        nc.vector.tensor_tensor(a1[:], fy[:], my1[:], ALU.mult)
        b0 = coord.tile([128, ncol], F32, tag=f"b0{ncol}")
        nc.vector.tensor_scalar(b0[:], fx[:], -1.0, 1.0, ALU.mult, ALU.add)
        nc.vector.tensor_tensor(b0[:], b0[:], mx0[:], ALU.mult)
        b1 = coord.tile([128, ncol], F32, tag=f"b1{ncol}")
        nc.vector.tensor_tensor(b1[:], fx[:], mx1[:], ALU.mult)

        nc.vector.tensor_tensor(w4dst[:, 0, :], a0[:], b0[:], ALU.mult)
        nc.vector.tensor_tensor(w4dst[:, 1, :], a0[:], b1[:], ALU.mult)
        nc.vector.tensor_tensor(w4dst[:, 2, :], a1[:], b0[:], ALU.mult)
        nc.vector.tensor_tensor(w4dst[:, 3, :], a1[:], b1[:], ALU.mult)

        pixf = coord.tile([128, ncol], F32, tag=f"pixf{ncol}")
        nc.vector.scalar_tensor_tensor(pixf[:], y0[:], 64.0, x0[:],
                                       ALU.mult, ALU.add)
        pt = coord.tile([128, ncol], F32, tag=f"pt{ncol}")
        nc.vector.tensor_scalar(pt[:], pixf[:], -128.0, 4222.0, ALU.max, ALU.min)
        nc.vector.tensor_scalar(pt[:], pt[:], 128.0, None, ALU.add)
        nc.vector.tensor_copy(pixdst, pt[:])

    # ---------------- prologue (own PSUM scope) ----------------
    with tc.tile_pool(name="prepsum", bufs=2, space="PSUM") as pps:
        # xT build (fp16 transpose) -> write both xpair slots directly
        # xpair[r, 0, :] = xrow[r - 128]; xpair[r, 1, :] = xrow[r - 64]
        # (xrow[i] = image pixel row i // 64, col i % 64; tile i covers
        #  xrows 128i..128i+127, written at slot1 r=64+128i, slot0 r=128+128i)
        for i in range(NBLK):
            psx = pps.tile([128, 128], F16, tag="psx")
            r0 = (2 * i + 1) * 64
            nc.tensor.transpose(psx[:], xsh[:, 1, r0:r0 + 128], ident16[:])
            xts = evac.tile([128, 128], F16, tag="xts")
            nc.scalar.activation(xts[:], psx[:], ACTF.Copy)
            nc.sync.dma_start(
                out=bass.AP(xpair_flat.tensor, 256 * (GUARD + 128 * i) + 128,
                            [[256, 128], [1, 128]]),
                in_=xts[:])
            nc.sync.dma_start(
                out=bass.AP(xpair_flat.tensor, 256 * (2 * GUARD + 128 * i),
                            [[256, 128], [1, 128]]),
                in_=xts[:])

        # offset conv (fp16 in, fp32 psum)
        def conv_tile(nb):
            ps = pps.tile([18, 512], F32, tag="psoff")
            for k in range(NTAP):
                ky, kx = k // 3, k % 3
                r0 = (nb * 8 + ky) * 64
                rhs = xsh[:, kx, r0:r0 + 512]
                nc.tensor.matmul(ps[:], wofft_sb[:, k, :], rhs,
                                 start=(k == 0), stop=(k == NTAP - 1))
            nc.scalar.activation(offsb[:, nb * 512:(nb + 1) * 512], ps[:], ACTF.Copy)

        for nb in range(4):
            conv_tile(nb)

        # --- early path: tap 0, half 0 -> first gather starts ASAP ---
        for i in range(16):
            pst = pps.tile([128, 18], F32, tag="pst")
            nc.tensor.transpose(pst[:], offsb[:, i * 128:(i + 1) * 128],
                                ident32[0:18, 0:18])
            nc.scalar.activation(offT_e[:, i, :], pst[:], ACTF.Copy)
        coord_block(offT_e[:, :, 0], offT_e[:, :, 1],
                    basepy_sb[:, 0, 0:16], basepx_sb[:, 0, 0:16],
                    w4_e, pix_e[:], 16)
        nc.sync.dma_start(out=io["pixb_e"], in_=pix_e[:])
        pixbe_flat = io["pixb_e"].rearrange("p n -> (p n)")
        tmpe = coord.tile([16, 8, 16], I16, tag="tmpe")
        nc.sync.dma_start(out=tmpe[:], in_=bass.AP(
            pixbe_flat.tensor, 0, [[16, 16], [256, 8], [1, 16]]))
        nc.vector.tensor_copy(
            idxw_e[0:16, :].rearrange("q (i h) -> q i h", h=8),
            tmpe[:].rearrange("q h i -> q i h"))
        for g in range(1, 8):
            nc.scalar.dma_start(out=idxw_e[16 * g:16 * (g + 1), :],
                                in_=idxw_e[0:16, :])

        for nb in range(4, 8):
            conv_tile(nb)

        # transpose offsets -> offT
        for i in range(NBLK):
            pst = pps.tile([128, 18], F32, tag="pst")
            nc.tensor.transpose(pst[:], offsb[:, i * 128:(i + 1) * 128],
                                ident32[0:18, 0:18])
            nc.scalar.activation(offT[:, i, :], pst[:], ACTF.Copy)


    # ---------------- coords / weights / indices (DVE) ----------------
    for k in range(NTAP):
        coord_block(offT[:, :, 2 * k], offT[:, :, 2 * k + 1],
                    basepy_sb[:, k, :], basepx_sb[:, k, :],
                    w4s[k], pix16[:, k, :], 32)

        # ---- per-tap idx wrap: DRAM bounce with big descriptors ----
        # pixb[p, 32k+i] = pix; wrap target idxw[q=p%16, s=i*8+p//16].
        # Stage 1 reads [q, p16, i] (contiguous 64B i-runs), stage 2
        # permutes (p16, i)->(i, p16) on DVE, stage 3 replicates to all
        # 128 partitions for the gather ucode.
        nc.sync.dma_start(out=pixb[:, 32 * k:32 * (k + 1)], in_=pix16[:, k, :])
        pixb_flat = pixb.rearrange("p n -> (p n)")
        tmpw = coord.tile([16, 8, 32], I16, tag="tmpw")
        src1 = bass.AP(pixb_flat.tensor, 32 * k,
                       [[288, 16], [16 * 288, 8], [1, 32]])
        nc.sync.dma_start(out=tmpw[:], in_=src1)
        idxw = idxws[k]
        nc.vector.tensor_copy(
            idxw[0:16, :].rearrange("q (i h) -> q i h", h=8),
            tmpw[:].rearrange("q h i -> q i h"))
        for g in range(1, 8):
            nc.scalar.dma_start(out=idxw[16 * g:16 * (g + 1), :],
                                in_=idxw[0:16, :])

    # ---------------- main loop ----------------
    gather_src = bass.AP(xpair_flat.tensor, 0, [[256, 4351], [1, 512]])

    with tc.tile_pool(name="psout", bufs=1, space="PSUM") as psout, \
         tc.tile_pool(name="pstr", bufs=3, space="PSUM") as pstr, \
         tc.tile_pool(name="gpool", bufs=3) as gpool, \
         tc.tile_pool(name="spool", bufs=2) as spool, \
         tc.tile_pool(name="tpool", bufs=4) as tpool:
        for hf in range(NHALF):
            blk0 = hf * BPH
            pso = [psout.tile([128, 512], F32, tag=f"pso{c}", name=f"pso{c}_{hf}")
                   for c in range(4)]
            for k in range(NTAP):
                gq = gpool.tile([128, BPH, 512], F16, tag="gq")
                early = (hf == 0 and k == 0)
                if not CFG["gather"]:
                    nc.vector.memset(gq[:], 0.25)
                else:
                    nc.gpsimd.dma_gather(
                        out_ap=gq[:],
                        in_ap=gather_src,
                        idxs_ap=(idxw_e[:, :] if early else
                                 idxws[k][:, blk0 * 8:(blk0 + BPH) * 8]),
                        num_idxs=BPH * 128,
                        num_idxs_reg=BPH * 128,
                        elem_size=512,
                        elem_step=256,
                        single_packet=False,
                    )
                sampT = spool.tile([128, BPH, 128], F16, tag="sampT")
                for i in range(BPH):
                    if not CFG["combine"]:
                        nc.vector.tensor_copy(sampT[:, i, :], gq[:, i, 0:128])
                        continue
                    ib = blk0 + i
                    wsrc, iw = (w4_e, i) if early else (w4s[k], ib)
                    t01 = tpool.tile([128, 128], F16, tag="t01")
                    nc.vector._custom_dve(
                        SCALE2, out=t01[:], in0=gq[:, i, 0:128],
                        in1=gq[:, i, 128:256],
                        s0=wsrc[:, 0, iw:iw + 1], s1=wsrc[:, 2, iw:iw + 1])
                    t23 = tpool.tile([128, 128], F16, tag="t23")
                    nc.vector._custom_dve(
                        SCALE2, out=t23[:], in0=gq[:, i, 256:384],
                        in1=gq[:, i, 384:512],
                        s0=wsrc[:, 1, iw:iw + 1], s1=wsrc[:, 3, iw:iw + 1])
                    nc.vector.tensor_tensor(sampT[:, i, :], t01[:], t23[:],
                                            ALU.add)
                sampN = spool.tile([128, BPH * 128], F16, tag="sampN")
                for i4 in range(BPH // 4):
                    if not CFG["transpose"]:
                        for i in range(4 * i4, 4 * i4 + 4):
                            nc.vector.tensor_copy(sampN[:, i * 128:(i + 1) * 128], sampT[:, i, :])
                        continue
                    pss = pstr.tile([128, 512], F16, tag="pss")
                    for j in range(4):
                        i = 4 * i4 + j
                        nc.tensor.transpose(pss[:, j * 128:(j + 1) * 128],
                                            sampT[:, i, :], ident16[:])
                    nc.scalar.activation(sampN[:, i4 * 512:(i4 + 1) * 512], pss[:],
                                         ACTF.Copy)
                if io["debug"] and hf == 0 and k == 0:
                    nc.sync.dma_start(out=io["d_sampT"], in_=sampT[:].rearrange("p a b -> p (a b)"))
                    nc.sync.dma_start(out=io["d_sampN"], in_=sampN[:])
                for c in (range(4) if CFG["matmul"] else ()):
                    nc.tensor.matmul(pso[c][:], wmainT_sb[:, k, :],
                                     sampN[:, c * 512:(c + 1) * 512],
                                     start=(k == 0), stop=(k == NTAP - 1))
            for c in range(4):
                osb = evac.tile([128, 512], F32, tag="osb")
                nc.scalar.activation(osb[:], pso[c][:], ACTF.Copy)
                l0 = hf * 2048 + c * 512
                nc.sync.dma_start(out=out[:, l0:l0 + 512], in_=osb[:])
    ctx.close()


# ======================= runner =======================
import concourse.bacc as _bacc
from concourse.bass_utils import run_bass_kernel_spmd as _run_spmd
from concourse.bass_interp import get_hw_module as _get_hw_module

_MODULE_CACHE = {}


def _get_module(num_cores):
    key = num_cores
    if key not in _MODULE_CACHE:
        nc = _bacc.Bacc("TRN2", target_bir_lowering=False, debug=False,
                        enable_asserts=False, num_devices=num_cores)
        io = declare_io(nc, debug=False)
        with tile.TileContext(nc) as tc:
            build(tc, io)
        nc.compile()
        nc.m = _get_hw_module(nc.m)
        _MODULE_CACHE[key] = nc
    return _MODULE_CACHE[key]


def kernel(x, w_offset, w):
    """DeformConv: x [8,128,64,64] f32, w_offset [18,128,3,3] f32,
    w [128,128,3,3] f32 -> out [8,128,64,64] f32. One sample per NeuronCore."""
    x = np.ascontiguousarray(np.asarray(x), dtype=np.float32)
    w_offset = np.asarray(w_offset)
    w = np.asarray(w)
    B = x.shape[0]
    nc = _get_module(B)
    shared = {**host_weights(w_offset, w), **host_constants()}
    in_maps = [{"xin": x[b].reshape(128, HW), **shared} for b in range(B)]
    res = _run_spmd(nc, in_maps, core_ids=list(range(B)))
    out = np.stack([res.results[b]["out"].reshape(128, 64, 64) for b in range(B)])
    return out.astype(np.float32)



# revision 20
# speedup vs baseline: 1.2281x; 1.2281x over previous
"""Bass/Tile kernel builder for DeformConv (one sample per NeuronCore).

Index conventions:
  l = i * 128 + p   (p = SBUF partition, i = l-block 0..31)
  image row = l // 64, col = l % 64;  tap k = ky*3 + kx
  base arrays [128 p, 9 k, 32 i] fp32
  corner weights w4 [128, 4 r, 9 k, 32 i] fp32
  pix16 [128, 18 t, 32 i] int16, t = 2k + (0 top / 1 bottom pair)
  xtd DRAM [4224, 128] fp16, row = pixel + 64 (guard bands zeroed)
  gather t: list position j = i_local*128 + p -> idx[q=j%16, s=j//16=i_local*8+p//16]
"""
from contextlib import ExitStack

import numpy as np

import concourse.bass as bass
import concourse.mybir as mybir
import concourse.tile as tile
from concourse import masks
from concourse import dve_ops as _dve_ops


def _register_scale2_add():
    """Custom DVE op: out = in0*s0 + in1*s1 (two per-partition scalars).
    Does a corner-pair of the bilinear combine in one Vector instruction."""
    for op in _dve_ops.OPS:
        if op.name == "SCALE2_ADD_ANT":
            return op
    from concourse.dve_spec import Spec, Src0, Src1, C0, C1, lower
    from concourse.dve_uop import DveOpSpec

    spec = Spec(
        body=Src0 * C0 + Src1 * C1,
        reference=lambda in0, in1, s0, s1, imm2: (
            in0.astype(np.float32) * s0 + in1.astype(np.float32) * s1),
    )
    shas = {}
    for ver in ("v3", "v4"):
        s = DveOpSpec(name="SCALE2_ADD_ANT", opcode=0,
                      uops=lower(spec, ver=ver), rd1_en=True)
        shas[ver] = s.sha(ver)
    op = _dve_ops.DveOp("SCALE2_ADD_ANT", spec, subdim=False, uops_sha=shas)
    _dve_ops.OPS.append(op)
    _dve_ops._SUB_OPCODE_FOR_NAME[op.name] = (
        _dve_ops._CUSTOM_DVE_ROW_BASE + len(_dve_ops.OPS) - 1)
    _dve_ops.CUSTOM_DVE_SPECS[op.name] = spec
    return op


SCALE2 = _register_scale2_add()

F32 = mybir.dt.float32
F32R = mybir.dt.float32r
F16 = mybir.dt.float16
I16 = mybir.dt.int16
ALU = mybir.AluOpType
ACTF = mybir.ActivationFunctionType

Cin = Cout = 128
HW = 4096
NTAP = 9
GUARD = 64
NROWS = HW + 2 * GUARD
NBLK = 32
NHALF = 2
BPH = NBLK // NHALF  # blocks per half

# stage toggles for cost attribution (dev only)
CFG = {"gather": True, "combine": True, "transpose": True, "matmul": True}


def host_constants():
    l = np.arange(HW)
    p = l % 128
    blk = l // 128
    i_img = l // 64
    j_img = l % 64
    ky = np.arange(9) // 3
    kx = np.arange(9) % 3
    basepy = np.zeros((128, 9, 32), np.float32)
    basepx = np.zeros((128, 9, 32), np.float32)
    for k in range(9):
        basepy[p, k, blk] = i_img - 1 + ky[k]
        basepx[p, k, blk] = j_img - 1 + kx[k]
    return {"basepy": basepy, "basepx": basepx}


def host_weights(w_offset, w):
    wofft = np.ascontiguousarray(
        w_offset.transpose(2, 3, 1, 0).reshape(9, 128, 18)).astype(np.float16)
    wmainT = np.ascontiguousarray(
        w.transpose(2, 3, 1, 0).reshape(9, 128, 128)).astype(np.float16)
    return {"wofft": wofft, "wmainT": wmainT}


def declare_io(nc, debug=False):
    io = {}
    io["xin"] = nc.dram_tensor("xin", (128, HW), F32, kind="ExternalInput").ap()
    io["wofft"] = nc.dram_tensor("wofft", (9, 128, 18), F16, kind="ExternalInput").ap()
    io["wmainT"] = nc.dram_tensor("wmainT", (9, 128, 128), F16, kind="ExternalInput").ap()
    io["basepy"] = nc.dram_tensor("basepy", (128, 9, 32), F32, kind="ExternalInput").ap()
    io["basepx"] = nc.dram_tensor("basepx", (128, 9, 32), F32, kind="ExternalInput").ap()
    io["out"] = nc.dram_tensor("out", (128, HW), F32, kind="ExternalOutput").ap()
    io["pixb"] = nc.dram_tensor("pixb", (128, 9 * 32), I16,
                                kind="ExternalOutput" if debug else "Internal").ap()
    io["pixb_e"] = nc.dram_tensor("pixb_e", (128, 16), I16, kind="Internal").ap()
    io["xpair"] = nc.dram_tensor("xpair", (4352, 2, 128), F16, kind="Internal").ap()
    io["debug"] = debug
    if debug:
        io["d_offsb"] = nc.dram_tensor("d_offsb", (18, HW), F32, kind="ExternalOutput").ap()
        io["d_offT"] = nc.dram_tensor("d_offT", (128, 32 * 18), F32, kind="ExternalOutput").ap()
        io["d_w4"] = nc.dram_tensor("d_w4", (128, 4 * 9 * 32), F32, kind="ExternalOutput").ap()
        io["d_idxw"] = nc.dram_tensor("d_idxw", (128, 9 * 256), I16, kind="ExternalOutput").ap()
        io["d_gtop"] = nc.dram_tensor("d_gtop", (128, BPH * 256), F16, kind="ExternalOutput").ap()
        io["d_gbot"] = nc.dram_tensor("d_gbot", (128, BPH * 256), F16, kind="ExternalOutput").ap()
        io["d_sampT"] = nc.dram_tensor("d_sampT", (128, BPH * 128), F16, kind="ExternalOutput").ap()
        io["d_sampN"] = nc.dram_tensor("d_sampN", (128, BPH * 128), F16, kind="ExternalOutput").ap()
        for nm in ("d_py", "d_fy", "d_y0", "d_x0", "d_fx", "d_my0", "d_a0"):
            io[nm] = nc.dram_tensor(nm, (128, 32), F32, kind="ExternalOutput").ap()
    return io


def build(tc: tile.TileContext, io: dict):
    nc = tc.nc
    xin, wofft, wmainT = io["xin"], io["wofft"], io["wmainT"]
    basepy, basepx, out = io["basepy"], io["basepx"], io["out"]
    pixb, xpair = io["pixb"], io["xpair"]

    ctx = ExitStack()
    const = ctx.enter_context(tc.tile_pool(name="const", bufs=1))
    persist = ctx.enter_context(tc.tile_pool(name="persist", bufs=1))
    coord = ctx.enter_context(tc.tile_pool(name="coord", bufs=2))
    evac = ctx.enter_context(tc.tile_pool(name="evac", bufs=3))

    ident32 = const.tile([128, 128], F32)
    masks.make_identity(nc, ident32[:])
    ident16 = const.tile([128, 128], F16)
    masks.make_identity(nc, ident16[:])
    zeros16 = const.tile([128, 128], F16)
    nc.vector.memset(zeros16[:], 0.0)

    # zero the xpair cells no data write covers (guard bands)
    xpair_flat = xpair.rearrange("r s c -> (r s c)")
    for off, rows in (
        (0, 128), (256 * 4224, 128),             # slot0: r in [0,128)+[4224,4352)
        (128, 64),                               # slot1: r in [0,64)
        (256 * 4160 + 128, 128), (256 * 4288 + 128, 64),  # slot1: [4160,4352)
    ):
        nc.sync.dma_start(
            out=bass.AP(xpair_flat.tensor, off, [[256, rows], [1, 128]]),
            in_=zeros16[0:rows, :])

    xpad = persist.tile([128, 66, 66], F16)
    nc.vector.memset(xpad[:], 0.0)
    nc.gpsimd.dma_start(out=xpad[:, 1:65, 1:65],
                        in_=xin.rearrange("c (h w) -> c h w", h=64))
    # column-shifted contiguous copies: xsh[:, kx, r, j] = xpad[c, r, j+kx]
    xsh = persist.tile([128, 3, 66 * 64], F16)
    for kx in range(3):
        nc.vector.tensor_copy(
            xsh[:, kx, :].rearrange("p (r j) -> p r j", r=66),
            xpad[:, :, kx:kx + 64])

    wofft_sb = persist.tile([128, 9, 18], F16)
    nc.sync.dma_start(out=wofft_sb[:], in_=wofft.rearrange("k c f -> c k f"))
    wmainT_sb = persist.tile([128, 9, 128], F16)
    nc.sync.dma_start(out=wmainT_sb[:], in_=wmainT.rearrange("k c o -> c k o"))
    basepy_sb = persist.tile([128, 9, 32], F32)
    nc.sync.dma_start(out=basepy_sb[:], in_=basepy)
    basepx_sb = persist.tile([128, 9, 32], F32)
    nc.sync.dma_start(out=basepx_sb[:], in_=basepx)

    offsb = persist.tile([18, HW], F32)
    offT = persist.tile([128, 32, 18], F32)
    w4s = [persist.tile([128, 4, 32], F32, name=f"w4_{k}") for k in range(NTAP)]
    pix16 = persist.tile([128, 9, 32], I16)
    idxws = [persist.tile([128, 256], I16, name=f"idxw_{k}") for k in range(NTAP)]

    def coord_block(dy, dx, bpy, bpx, w4dst, pixdst, ncol):
        """bilinear coords -> 4 corner weights + clamped pair-row index."""
        py = coord.tile([128, ncol], F32, tag=f"py{ncol}")
        nc.vector.tensor_tensor(py[:], dy, bpy, ALU.add)
        px = coord.tile([128, ncol], F32, tag=f"px{ncol}")
        nc.vector.tensor_tensor(px[:], dx, bpx, ALU.add)

        def floorfrac(s, tagp):
            ti = coord.tile([128, ncol], mybir.dt.int32, tag=tagp + "i")
            nc.vector.tensor_copy(ti[:], s[:])
            tf = coord.tile([128, ncol], F32, tag=tagp + "f")
            nc.vector.tensor_copy(tf[:], ti[:])
            gt = coord.tile([128, ncol], F32, tag=tagp + "g")
            nc.vector.tensor_tensor(gt[:], tf[:], s[:], ALU.is_gt)
            fl = coord.tile([128, ncol], F32, tag=tagp + "fl")
            nc.vector.tensor_tensor(fl[:], tf[:], gt[:], ALU.subtract)
            fr = coord.tile([128, ncol], F32, tag=tagp + "fr")
            nc.vector.tensor_tensor(fr[:], s[:], fl[:], ALU.subtract)
            return fl, fr

        y0, fy = floorfrac(py, f"yy{ncol}")
        x0, fx = floorfrac(px, f"xx{ncol}")

        def wmask(s, lo, hi, tag):
            m1 = coord.tile([128, ncol], F32, tag=tag + "a")
            nc.vector.tensor_scalar(m1[:], s[:], float(lo), None, ALU.is_ge)
            m2 = coord.tile([128, ncol], F32, tag=tag + "b")
            nc.vector.tensor_scalar(m2[:], s[:], float(hi), None, ALU.is_le)
            m = coord.tile([128, ncol], F32, tag=tag)
            nc.vector.tensor_tensor(m[:], m1[:], m2[:], ALU.mult)
            return m

        my0 = wmask(y0, 0, 63, f"my0{ncol}")
        my1 = wmask(y0, -1, 62, f"my1{ncol}")
        mx0 = wmask(x0, 0, 63, f"mx0{ncol}")
        mx1 = wmask(x0, -1, 62, f"mx1{ncol}")

        a0 = coord.tile([128, ncol], F32, tag=f"a0{ncol}")
        nc.vector.tensor_scalar(a0[:], fy[:], -1.0, 1.0, ALU.mult, ALU.add)
        nc.vector.tensor_tensor(a0[:], a0[:], my0[:], ALU.mult)
        a1 = coord.tile([128, ncol], F32, tag=f"a1{ncol}")
        nc.vector.tensor_tensor(a1[:], fy[:], my1[:], ALU.mult)
        b0 = coord.tile([128, ncol], F32, tag=f"b0{ncol}")
        nc.vector.tensor_scalar(b0[:], fx[:], -1.0, 1.0, ALU.mult, ALU.add)
        nc.vector.tensor_tensor(b0[:], b0[:], mx0[:], ALU.mult)
        b1 = coord.tile([128, ncol], F32, tag=f"b1{ncol}")
        nc.vector.tensor_tensor(b1[:], fx[:], mx1[:], ALU.mult)

        nc.vector.tensor_tensor(w4dst[:, 0, :], a0[:], b0[:], ALU.mult)
        nc.vector.tensor_tensor(w4dst[:, 1, :], a0[:], b1[:], ALU.mult)
        nc.vector.tensor_tensor(w4dst[:, 2, :], a1[:], b0[:], ALU.mult)
        nc.vector.tensor_tensor(w4dst[:, 3, :], a1[:], b1[:], ALU.mult)

        pixf = coord.tile([128, ncol], F32, tag=f"pixf{ncol}")
        nc.vector.scalar_tensor_tensor(pixf[:], y0[:], 64.0, x0[:],
                                       ALU.mult, ALU.add)
        pt = coord.tile([128, ncol], F32, tag=f"pt{ncol}")
        nc.vector.tensor_scalar(pt[:], pixf[:], -128.0, 4222.0, ALU.max, ALU.min)
        nc.vector.tensor_scalar(pt[:], pt[:], 128.0, None, ALU.add)
        nc.vector.tensor_copy(pixdst, pt[:])

    # ---------------- prologue (own PSUM scope) ----------------
    with tc.tile_pool(name="prepsum", bufs=2, space="PSUM") as pps:

        # offset conv (fp16 in, fp32 psum)
        def conv_tile(nb):
            ps = pps.tile([18, 512], F32, tag="psoff")
            for k in range(NTAP):
                ky, kx = k // 3, k % 3
                r0 = (nb * 8 + ky) * 64
                rhs = xsh[:, kx, r0:r0 + 512]
                nc.tensor.matmul(ps[:], wofft_sb[:, k, :], rhs,
                                 start=(k == 0), stop=(k == NTAP - 1))
            nc.scalar.activation(offsb[:, nb * 512:(nb + 1) * 512], ps[:], ACTF.Copy)

        for nb in range(8):
            conv_tile(nb)

        # transpose offsets -> offT
        for i in range(NBLK):
            pst = pps.tile([128, 18], F32, tag="pst")
            nc.tensor.transpose(pst[:], offsb[:, i * 128:(i + 1) * 128],
                                ident32[0:18, 0:18])
            nc.scalar.activation(offT[:, i, :], pst[:], ACTF.Copy)


        # xT build (fp16 transpose) -> write both xpair slots directly
        # xpair[r, 0, :] = xrow[r - 128]; xpair[r, 1, :] = xrow[r - 64]
        # (xrow[i] = image pixel row i // 64, col i % 64; tile i covers
        #  xrows 128i..128i+127, written at slot1 r=64+128i, slot0 r=128+128i)
        for i in range(NBLK):
            psx = pps.tile([128, 128], F16, tag="psx")
            r0 = (2 * i + 1) * 64
            nc.tensor.transpose(psx[:], xsh[:, 1, r0:r0 + 128], ident16[:])
            xts = evac.tile([128, 128], F16, tag="xts")
            nc.scalar.activation(xts[:], psx[:], ACTF.Copy)
            nc.sync.dma_start(
                out=bass.AP(xpair_flat.tensor, 256 * (GUARD + 128 * i) + 128,
                            [[256, 128], [1, 128]]),
                in_=xts[:])
            nc.sync.dma_start(
                out=bass.AP(xpair_flat.tensor, 256 * (2 * GUARD + 128 * i),
                            [[256, 128], [1, 128]]),
                in_=xts[:])

    # ---------------- coords / weights / indices (DVE) ----------------
    for k in range(NTAP):
        coord_block(offT[:, :, 2 * k], offT[:, :, 2 * k + 1],
                    basepy_sb[:, k, :], basepx_sb[:, k, :],
                    w4s[k], pix16[:, k, :], 32)

        # ---- per-tap idx wrap: DRAM bounce with big descriptors ----
        # pixb[p, 32k+i] = pix; wrap target idxw[q=p%16, s=i*8+p//16].
        # Stage 1 reads [q, p16, i] (contiguous 64B i-runs), stage 2
        # permutes (p16, i)->(i, p16) on DVE, stage 3 replicates to all
        # 128 partitions for the gather ucode.
        nc.sync.dma_start(out=pixb[:, 32 * k:32 * (k + 1)], in_=pix16[:, k, :])
        pixb_flat = pixb.rearrange("p n -> (p n)")
        tmpw = coord.tile([16, 8, 32], I16, tag="tmpw")
        src1 = bass.AP(pixb_flat.tensor, 32 * k,
                       [[288, 16], [16 * 288, 8], [1, 32]])
        nc.sync.dma_start(out=tmpw[:], in_=src1)
        idxw = idxws[k]
        nc.vector.tensor_copy(
            idxw[0:16, :].rearrange("q (i h) -> q i h", h=8),
            tmpw[:].rearrange("q h i -> q i h"))
        for g in range(1, 8):
            nc.scalar.dma_start(out=idxw[16 * g:16 * (g + 1), :],
                                in_=idxw[0:16, :])

    # ---------------- main loop ----------------
    gather_src = bass.AP(xpair_flat.tensor, 0, [[256, 4351], [1, 512]])

    with tc.tile_pool(name="psout", bufs=1, space="PSUM") as psout, \
         tc.tile_pool(name="pstr", bufs=3, space="PSUM") as pstr, \
         tc.tile_pool(name="gpool", bufs=3) as gpool, \
         tc.tile_pool(name="spool", bufs=2) as spool, \
         tc.tile_pool(name="tpool", bufs=4) as tpool:
        for hf in range(NHALF):
            blk0 = hf * BPH
            pso = [psout.tile([128, 512], F32, tag=f"pso{c}", name=f"pso{c}_{hf}")
                   for c in range(4)]
            for k in range(NTAP):
                gq = gpool.tile([128, BPH, 512], F16, tag="gq")
                if not CFG["gather"]:
                    nc.vector.memset(gq[:], 0.25)
                else:
                    nc.gpsimd.dma_gather(
                        out_ap=gq[:],
                        in_ap=gather_src,
                        idxs_ap=idxws[k][:, blk0 * 8:(blk0 + BPH) * 8],
                        num_idxs=BPH * 128,
                        num_idxs_reg=BPH * 128,
                        elem_size=512,
                        elem_step=256,
                        single_packet=False,
                    )
                sampT = spool.tile([128, BPH, 128], F16, tag="sampT")
                for i in range(BPH):
                    if not CFG["combine"]:
                        nc.vector.tensor_copy(sampT[:, i, :], gq[:, i, 0:128])
                        continue
                    ib = blk0 + i
                    wsrc, iw = w4s[k], ib
                    t01 = tpool.tile([128, 128], F16, tag="t01")
                    nc.vector._custom_dve(
                        SCALE2, out=t01[:], in0=gq[:, i, 0:128],
                        in1=gq[:, i, 128:256],
                        s0=wsrc[:, 0, iw:iw + 1], s1=wsrc[:, 2, iw:iw + 1])
                    t23 = tpool.tile([128, 128], F16, tag="t23")
                    nc.vector._custom_dve(
                        SCALE2, out=t23[:], in0=gq[:, i, 256:384],
                        in1=gq[:, i, 384:512],
                        s0=wsrc[:, 1, iw:iw + 1], s1=wsrc[:, 3, iw:iw + 1])
                    nc.vector.tensor_tensor(sampT[:, i, :], t01[:], t23[:],
                                            ALU.add)
                sampN = spool.tile([128, BPH * 128], F16, tag="sampN")
                for i4 in range(BPH // 4):
                    if not CFG["transpose"]:
                        for i in range(4 * i4, 4 * i4 + 4):
                            nc.vector.tensor_copy(sampN[:, i * 128:(i + 1) * 128], sampT[:, i, :])
                        continue
                    pss = pstr.tile([128, 512], F16, tag="pss")
                    for j in range(4):
                        i = 4 * i4 + j
                        nc.tensor.transpose(pss[:, j * 128:(j + 1) * 128],
                                            sampT[:, i, :], ident16[:])
                    nc.scalar.activation(sampN[:, i4 * 512:(i4 + 1) * 512], pss[:],
                                         ACTF.Copy)
                if io["debug"] and hf == 0 and k == 0:
                    nc.sync.dma_start(out=io["d_sampT"], in_=sampT[:].rearrange("p a b -> p (a b)"))
                    nc.sync.dma_start(out=io["d_sampN"], in_=sampN[:])
                for c in (range(4) if CFG["matmul"] else ()):
                    nc.tensor.matmul(pso[c][:], wmainT_sb[:, k, :],
                                     sampN[:, c * 512:(c + 1) * 512],
                                     start=(k == 0), stop=(k == NTAP - 1))
            for c in range(4):
                osb = evac.tile([128, 512], F32, tag="osb")
                nc.scalar.activation(osb[:], pso[c][:], ACTF.Copy)
                l0 = hf * 2048 + c * 512
                nc.sync.dma_start(out=out[:, l0:l0 + 512], in_=osb[:])
    ctx.close()


# ======================= runner =======================
import concourse.bacc as _bacc
from concourse.bass_utils import run_bass_kernel_spmd as _run_spmd
from concourse.bass_interp import get_hw_module as _get_hw_module

_MODULE_CACHE = {}


def _get_module(num_cores):
    key = num_cores
    if key not in _MODULE_CACHE:
        nc = _bacc.Bacc("TRN2", target_bir_lowering=False, debug=False,
                        enable_asserts=False, num_devices=num_cores)
        io = declare_io(nc, debug=False)
        with tile.TileContext(nc) as tc:
            build(tc, io)
        nc.compile()
        nc.m = _get_hw_module(nc.m)
        _MODULE_CACHE[key] = nc
    return _MODULE_CACHE[key]


def kernel(x, w_offset, w):
    """DeformConv: x [8,128,64,64] f32, w_offset [18,128,3,3] f32,
    w [128,128,3,3] f32 -> out [8,128,64,64] f32. One sample per NeuronCore."""
    x = np.ascontiguousarray(np.asarray(x), dtype=np.float32)
    w_offset = np.asarray(w_offset)
    w = np.asarray(w)
    B = x.shape[0]
    nc = _get_module(B)
    shared = {**host_weights(w_offset, w), **host_constants()}
    in_maps = [{"xin": x[b].reshape(128, HW), **shared} for b in range(B)]
    res = _run_spmd(nc, in_maps, core_ids=list(range(B)))
    out = np.stack([res.results[b]["out"].reshape(128, 64, 64) for b in range(B)])
    return out.astype(np.float32)

